# revision 17
# baseline (speedup 1.0000x reference)
"""MinLSTM Trainium2 kernel (fp8-DoubleRow edition).

Full-input contract: kernel(**inputs) takes the complete (unsharded) numpy
inputs of the reference model and returns the full [B, T+1, H] float32 output.

Math (identical to the reference's log-space scan, computed in linear space):
    a = x @ W_f ;  b = x @ W_i ;  c = x @ W_h       (zero biases asserted)
    f = sigmoid(a) / (sigmoid(a) + sigmoid(b))      # forget gate
    i = 1 - f                                       # input gate
    g = max(c + 0.5, sigmoid(c))                    # = exp(log_g(c))
    h_t = f_t * h_{t-1} + i_t * g_t,   h_{-1} = g(h_0)

Matmul scheme (all fp8 e4m3 with DoubleRow perf mode, 2 k-tiles/instr at
0.5 cycles/row = 4x the f32r rate). Weights are scaled by 64 so the W
residual of the h-projection is representable in e4m3; every PSUM slice
holds 64*(pre + 0.5) and one sigmoid pass applies scale=1/64, bias=-0.5:
    a_psum = ones*0.125(bias=+32) + x8 @ q8(64 W_f)            (plain)
    b_psum = ones*0.125(bias=+32) + x8 @ q8(64 W_i)            (plain)
    c_psum = x8 @ Whi + x8 @ Elo + xr @ Wxr                    (corrected)
        Whi = q8(64 W_h); Elo = q8(64 W_h - Whi)               (W residual)
        xr  = q8(x - x8) with row 511 := 1.0                   (x residual)
        Wxr = Whi with row 511 := 32.0                         (c's +32 bias)
The c-projection carries both residual corrections because h is ~1:1
sensitive to g but only ~0.3x to f (measured): end-to-end l2 ~ 6.5e-3.

Per-core engine placement (GPSIMD can't touch PSUM or TensorScalarPtr ops):
    ACT : one sigmoid pass per (chunk, ht) over [128, 3, TC] PSUM
    DVE : fused f = sa/(sa+sb) custom op (bit-NOT recip, deg-1 poly),
          g = max(c+0.5, sigma_c) stt from PSUM, w = 1-f tensor_scalar,
          tensor_tensor_scan (fp32 state, f16 data)
    Pool: v = w * g tensor_tensor
    PE  : 11 DoubleRow matmuls per (chunk, ht)

Sharding: 8 cores, core c -> (sample b = c//2, H-half hh = c%2, 256 ch).
Fully independent cores, no collectives. Host packs x into fp8 pair-layout
streams [128, kp, i, T]; host assembles the output (f16 -> f32).
"""

from contextlib import ExitStack

import numpy as np
import ml_dtypes

import concourse.bacc as bacc
import concourse.tile as tile
import concourse.mybir as mybir
from concourse.bass_utils import run_bass_kernel_spmd

import concourse.dve_ops as _dve_ops
from concourse.dve_spec import (Spec as _Spec, Src0 as _S0, Src1 as _S1,
                                C0 as _C0, C1 as _C1,
                                AluOp as _AluOp, Bin as _Bin, lower as _lower)
from concourse.dve_uop import DveOpSpec as _DveOpSpec
from concourse.dve_table_gen import dve_ver_for as _dve_ver_for

F8 = mybir.dt.float8e4
F16 = mybir.dt.float16
F32 = mybir.dt.float32
AF = mybir.ActivationFunctionType
OP = mybir.AluOpType
PM = mybir.MatmulPerfMode

NPF8 = ml_dtypes.float8_e4m3

# ---- fused custom DVE op: f = in0 / (in0 + in1) ---------------------------
# x = in0+in1 (positive); nx = bitcast(~x) seed; u = x*nx in [-4.5,-4);
# f = ((u*c0 + c1) * nx) * in0.  Deg-1 minimax on u: max rel err 2.9e-3.
FDIV_CONSTS = {"s0": -0.05551854, "s1": -0.47192850}


def _register_fdiv():
    name = "FRAC_SIGMOID_ANT"
    if name in _dve_ops._SUB_OPCODE_FOR_NAME:
        return next(o for o in _dve_ops.OPS if o.name == name)
    _x = _S0 + _S1
    _nx = _Bin(_AluOp.BITWISE_NOT, _x, _x)
    _u = _x * _nx
    _f = ((_u * _C0 + _C1) * _nx) * _S0

    def _ref(in0, in1, s0, s1):
        a = np.asarray(in0, np.float32)
        b = np.asarray(in1, np.float32)
        x = (a + b).astype(np.float32)
        nx = (~x.view(np.int32)).view(np.float32)
        u = (x * nx).astype(np.float32)
        return ((u * s0 + s1) * nx * a).astype(np.float32)

    spec = _Spec(body=_f, reference=_ref)
    row = _dve_ops._CUSTOM_DVE_ROW_BASE + len(_dve_ops.OPS)
    assert row < 0x20
    ver = _dve_ver_for("TRN2")
    sha = _DveOpSpec(name=name, opcode=row, uops=_lower(spec, ver=ver),
                     rd1_en=True).sha(ver)
    op = _dve_ops.DveOp(name, spec, subdim=False, uops_sha={ver: sha})
    _dve_ops.OPS.append(op)
    _dve_ops.CUSTOM_DVE_SPECS[name] = spec
    _dve_ops._SUB_OPCODE_FOR_NAME[name] = row
    return op


FDIV_OP = _register_fdiv()

B, T, D, H = 4, 8192, 512, 512
NCORES = 8
HS = H // 2          # 256 channels per core
NHT = 2              # 128-channel tiles per core
TC = 512             # T chunk width
UC = 2 * TC          # unit = 2 chunks
NU = T // UC         # 8 units
WSCALE = 64.0        # weight scale (keeps W residual out of e4m3 subnormals)

PSCAN_DEFER = 4
MM_MODE = "fp8"      # kept for test.py compatibility

_nc_cache = {}


def _build_nc(mm_mode=MM_MODE):
    nc = bacc.Bacc("TRN2", target_bir_lowering=False, debug=False,
                   num_devices=NCORES)
    # fp8 pair-layout x streams: [p, kp, i, t] ; d = kp*256 + i*128 + p
    x8d = nc.dram_tensor("x8", [128, 2, 2, T], F8, kind="ExternalInput")
    xrd = nc.dram_tensor("xr", [128, 2, 2, T], F8, kind="ExternalInput")
    # stationary: 21 pair-slices [p, sl, i, m]; per ht: wf0 wf1 wi0 wi1
    # whi0 whi1 elo0 elo1 wxr0 wxr1 ; slice 20 = bias 0.125
    wd = nc.dram_tensor("w", [128, 21, 2, 128], F8, kind="ExternalInput")
    onesd = nc.dram_tensor("ones", [128, 2, UC], F8, kind="ExternalInput")
    # aux cols: 0..1 = g(h_0) per ht ; 2 = -0.5 (sigmoid bias)
    auxd = nc.dram_tensor("aux", [128, 3], F32, kind="ExternalInput")
    outd = nc.dram_tensor("out", [128, NHT, T], F16, kind="ExternalOutput")

    with tile.TileContext(nc) as tc, ExitStack() as ctx:
        wpool = ctx.enter_context(tc.tile_pool(name="w", bufs=1))
        xpool = ctx.enter_context(tc.tile_pool(name="x", bufs=2))
        gpool = ctx.enter_context(tc.tile_pool(name="g", bufs=3))
        hpool = ctx.enter_context(tc.tile_pool(name="h", bufs=3))
        ppool = ctx.enter_context(tc.tile_pool(name="p", bufs=1, space="PSUM"))

        wt = wpool.tile([128, 21, 2, 128], F8, tag="w")
        nc.scalar.dma_start(wt[:], wd[:])
        onest = wpool.tile([128, 2, UC], F8, tag="ones")
        nc.scalar.dma_start(onest[:], onesd[:])
        auxt = wpool.tile([128, 3], F32, tag="aux")
        nc.scalar.dma_start(auxt[:], auxd[:])

        carry = [None] * NHT
        pending_out = []
        pending_scan = []
        units = {}

        def _emit_scan(ht, pu):
            pf, pv, ph = units[pu]
            ini = auxt[:, ht:ht + 1] if pu == 0 else carry[ht]
            nc.vector.tensor_tensor_scan(ph[ht][:], pf[ht][:], pv[ht][:], ini,
                                         OP.mult, OP.add)
            carry[ht] = ph[ht][:, UC - 1:UC]

        for u in range(NU):
            usl = slice(u * UC, (u + 1) * UC)
            x8t = xpool.tile([128, 2, 2, UC], F8, tag="x8", name="x8")
            xrt = xpool.tile([128, 2, 2, UC], F8, tag="xr", name="xr")
            if u == 0:
                # split the first loads per chunk so the pipeline fills early
                for k in range(2):
                    kx = slice(k * TC, (k + 1) * TC)
                    nc.sync.dma_start(x8t[:, :, :, kx], x8d[:, :, :, kx])
                    nc.scalar.dma_start(xrt[:, :, :, kx], xrd[:, :, :, kx])
            else:
                nc.sync.dma_start(x8t[:], x8d[:, :, :, usl])
                nc.scalar.dma_start(xrt[:], xrd[:, :, :, usl])

            sabg = [gpool.tile([128, 3, UC], F16, tag=f"s{ht}", name="sabg")
                    for ht in range(NHT)]
            g2 = [gpool.tile([128, UC], F16, tag=f"g{ht}", name="g2")
                  for ht in range(NHT)]
            f2u = [gpool.tile([128, UC], F16, tag=f"f{ht}", name="f2")
                   for ht in range(NHT)]
            w2u = [gpool.tile([128, UC], F16, tag=f"w{ht}", name="w2")
                   for ht in range(NHT)]
            v2u = [gpool.tile([128, UC], F16, tag=f"v{ht}", name="v2")
                   for ht in range(NHT)]
            h2u = [hpool.tile([128, UC], F16, tag=f"h{ht}", name="h2")
                   for ht in range(NHT)]
            units[u] = (f2u, v2u, h2u)
            # unit-wide pc: both chunks of the c-projection live at once so
            # sigma_c and g run once per unit at 1024 cols
            pcu = [ppool.tile([128, 2, TC], F32, tag=f"c{ht}", name=f"c{ht}")
                   for ht in range(NHT)]

            # chunk-major emission: the two ht chains interleave in every
            # in-order sequencer queue, so a stalled chunk of one chain never
            # blocks the ready chunk of the other
            for k in range(2):
                ksl = slice(k * TC, (k + 1) * TC)
                pab = [None] * NHT
                for ht in range(NHT):
                    W0 = ht * 10
                    pab[ht] = ppool.tile([128, 2, TC], F32, tag=f"p{ht}",
                                         name=f"p{ht}")
                    for kp in range(2):  # a, b projections
                        nc.tensor.matmul(pab[ht][:, 0, :],
                                         wt[:, W0 + kp, :, :],
                                         x8t[:, kp, :, ksl], start=(kp == 0),
                                         stop=(kp == 1),
                                         perf_mode=PM.DoubleRow)
                        nc.tensor.matmul(pab[ht][:, 1, :],
                                         wt[:, W0 + 2 + kp, :, :],
                                         x8t[:, kp, :, ksl], start=(kp == 0),
                                         stop=(kp == 1),
                                         perf_mode=PM.DoubleRow)
                    # c+0.5: x8@Whi + x8@Elo + xr@Wxr (Wxr row 511 = +32)
                    for j, (wsl, xt) in enumerate([(W0 + 4, x8t),
                                                   (W0 + 6, x8t),
                                                   (W0 + 8, xrt)]):
                        for kp in range(2):
                            nc.tensor.matmul(pcu[ht][:, k, :],
                                             wt[:, wsl + kp, :, :],
                                             xt[:, kp, :, ksl],
                                             start=(j == 0 and kp == 0),
                                             stop=(j == 2 and kp == 1),
                                             perf_mode=PM.DoubleRow)
                for ht in range(NHT):
                    nc.scalar.activation(sabg[ht][:, 0:2, ksl], pab[ht][:],
                                         AF.Sigmoid, scale=1.0 / 64.0)
            for ht in range(NHT):
                # sigma_c = sigmoid(pc/64 - 0.5) = sigmoid(c), whole unit
                sgc = sabg[ht][:, 2, :].rearrange("p (a b) -> p a b", a=2)
                nc.scalar.activation(sgc, pcu[ht][:], AF.Sigmoid,
                                     bias=auxt[:, 2:3], scale=1.0 / 64.0)
            for ht in range(NHT):
                # g = max(c + 0.5, sigma_c)   (PSUM read -> DVE)
                g3 = g2[ht][:].rearrange("p (a b) -> p a b", a=2)
                sg3 = sabg[ht][:, 2, :].rearrange("p (a b) -> p a b", a=2)
                nc.vector.scalar_tensor_tensor(g3, pcu[ht][:], 1.0 / 64.0,
                                               sg3, OP.mult, OP.max)
            for ht in range(NHT):
                # unit tail: f = sa/(sa+sb), w = 1-f, v = w*g
                nc.vector._custom_dve(FDIV_OP, out=f2u[ht][:],
                                      in0=sabg[ht][:, 0, :],
                                      in1=sabg[ht][:, 1, :],
                                      s0=FDIV_CONSTS["s0"],
                                      s1=FDIV_CONSTS["s1"])
                nc.vector.tensor_scalar(w2u[ht][:], f2u[ht][:], -1.0, 1.0,
                                        OP.mult, OP.add)
                nc.gpsimd.tensor_tensor(v2u[ht][:], w2u[ht][:], g2[ht][:],
                                        op=OP.mult)
                # scans are emitted one unit late so a scan waiting on
                # Pool's v never head-blocks the DVE queue
                pending_scan.append((ht, u))
            while len(pending_scan) > NHT:
                ph, pu = pending_scan.pop(0)
                _emit_scan(ph, pu)
            for ht in range(NHT):
                pending_out.append((ht, usl, h2u[ht]))
            # emit the previous unit's output DMAs here (one unit late, on
            # the ACT queue) so their scan-chain waits never sit in front of
            # the x prefetches or the sigma dispatches in a sequencer queue
            while len(pending_out) > NHT:
                oht, ousl, oh2 = pending_out.pop(0)
                nc.scalar.dma_start(outd[:, oht, ousl], oh2[:])
        while pending_scan:
            ph, pu = pending_scan.pop(0)
            _emit_scan(ph, pu)
        # final unit: split the store per chunk so the drain overlaps
        while pending_out:
            oht, ousl, oh2 = pending_out.pop(0)
            for k in range(2):
                osl = slice(ousl.start + k * TC, ousl.start + (k + 1) * TC)
                nc.scalar.dma_start(outd[:, oht, osl], oh2[:, k * TC:(k + 1) * TC])
    nc.compile()
    return nc


def _get_nc(mm_mode=MM_MODE):
    if mm_mode not in _nc_cache:
        _nc_cache[mm_mode] = _build_nc(mm_mode)
    return _nc_cache[mm_mode]


def _g_host(x):
    # exp(log_g(x)) of the reference, computed directly in fp32
    return np.where(x >= 0, x + 0.5, 1.0 / (1.0 + np.exp(-np.minimum(x, 0))))


def _q8(a):
    return a.astype(NPF8)


def _pack_x(xT_f32):
    """[D, T] f32 -> (x8 pack, xr pack) in [128, kp, i, T] fp8 pair layout;
    xr row d=511 is the constant 1.0 that delivers the c-gate's +32 bias."""
    x8 = _q8(xT_f32)
    xr = _q8(xT_f32 - x8.astype(np.float32))
    xr[511, :] = NPF8(1.0)
    def pack(a):
        return np.ascontiguousarray(
            a.reshape(2, 2, 128, T).transpose(2, 0, 1, 3))
    return pack(x8), pack(xr)


def _pack_w_slices(mat, ht):
    """[512, 256] fp8 -> two [128, 2, 128] pair slices (kp = 0, 1)."""
    r = mat.reshape(2, 2, 128, 2, 128)  # [kp, i, p, ht, m]
    return [np.ascontiguousarray(r[kp, :, :, ht, :].transpose(1, 0, 2))
            for kp in range(2)]


def _run(inputs, mm_mode=MM_MODE, trace=False):
    x = np.asarray(inputs["x"], np.float32)
    h_0 = np.asarray(inputs["h_0"], np.float32)
    W_f = np.asarray(inputs["W_f"], np.float32)
    b_f = np.asarray(inputs["b_f"], np.float32)
    W_i = np.asarray(inputs["W_i"], np.float32)
    b_i = np.asarray(inputs["b_i"], np.float32)
    W_h = np.asarray(inputs["W_h"], np.float32)
    b_h = np.asarray(inputs["b_h"], np.float32)
    assert (b_f == 0).all() and (b_i == 0).all() and (b_h == 0).all(), \
        "device program folds zero biases"

    g0 = _g_host(h_0[:, 0, :])  # [B, H]
    xpacks = [_pack_x(np.ascontiguousarray(x[b].T)) for b in range(B)]

    ones = np.ones((128, 2, UC), NPF8)
    in_maps = []
    for c in range(NCORES):
        b, hh = divmod(c, 2)
        hs = slice(hh * HS, (hh + 1) * HS)
        wf8 = _q8(WSCALE * W_f[:, hs])
        wi8 = _q8(WSCALE * W_i[:, hs])
        whi = _q8(WSCALE * W_h[:, hs])
        elo = _q8(WSCALE * W_h[:, hs] - whi.astype(np.float32))
        wxr = whi.copy()
        wxr[511, :] = NPF8(32.0)
        wcat = np.zeros((128, 21, 2, 128), NPF8)
        for ht in range(NHT):
            for mi, mat in enumerate((wf8, wi8, whi, elo, wxr)):
                s0, s1 = _pack_w_slices(mat, ht)
                wcat[:, ht * 10 + mi * 2, :, :] = s0
                wcat[:, ht * 10 + mi * 2 + 1, :, :] = s1
        wcat[:, 20, :, :] = NPF8(0.125)
        aux = np.empty((128, 3), np.float32)
        aux[:, 0:2] = g0[b, hs].reshape(2, 128).T
        aux[:, 2] = -0.5
        x8p, xrp = xpacks[b]
        in_maps.append({"x8": x8p, "xr": xrp, "w": wcat, "ones": ones,
                        "aux": aux})

    nc = _get_nc(mm_mode)
    res = run_bass_kernel_spmd(nc, in_maps, core_ids=list(range(NCORES)),
                               trace=trace)

    out = np.empty((B, T + 1, H), np.float32)
    out[:, 0, :] = g0
    for c in range(NCORES):
        b, hh = divmod(c, 2)
        hs = slice(hh * HS, (hh + 1) * HS)
        r = np.asarray(res.results[c]["out"], np.float32)  # [128, NHT, T]
        out[b, 1:, hs] = r.transpose(2, 1, 0).reshape(T, HS)
    return out, res


def kernel(**inputs):
    out, _ = _run(inputs)
    return out


# revision 23
# speedup vs baseline: 1.0702x; 1.0702x over previous
"""MinLSTM Trainium2 kernel (fp8-DoubleRow edition).

Full-input contract: kernel(**inputs) takes the complete (unsharded) numpy
inputs of the reference model and returns the full [B, T+1, H] float32 output.

Math (identical to the reference's log-space scan, computed in linear space):
    a = x @ W_f ;  b = x @ W_i ;  c = x @ W_h       (zero biases asserted)
    f = sigmoid(a) / (sigmoid(a) + sigmoid(b))      # forget gate
    i = 1 - f                                       # input gate
    g = max(c + 0.5, sigmoid(c))                    # = exp(log_g(c))
    h_t = f_t * h_{t-1} + i_t * g_t,   h_{-1} = g(h_0)

Matmul scheme (all fp8 e4m3 with DoubleRow perf mode, 2 k-tiles/instr at
0.5 cycles/row = 4x the f32r rate). Weights are scaled by 64 so the W
residual of the h-projection is representable in e4m3; every PSUM slice
holds 64*(pre + 0.5) and one sigmoid pass applies scale=1/64, bias=-0.5:
    a_psum = ones*0.125(bias=+32) + x8 @ q8(64 W_f)            (plain)
    b_psum = ones*0.125(bias=+32) + x8 @ q8(64 W_i)            (plain)
    c_psum = x8 @ Whi + x8 @ Elo + xr @ Wxr                    (corrected)
        Whi = q8(64 W_h); Elo = q8(64 W_h - Whi)               (W residual)
        xr  = q8(x - x8) with row 511 := 1.0                   (x residual)
        Wxr = Whi with row 511 := 32.0                         (c's +32 bias)
The c-projection carries both residual corrections because h is ~1:1
sensitive to g but only ~0.3x to f (measured): end-to-end l2 ~ 6.5e-3.

Per-core engine placement (GPSIMD can't touch PSUM or TensorScalarPtr ops):
    ACT : one sigmoid pass per (chunk, ht) over [128, 3, TC] PSUM
    DVE : fused f = sa/(sa+sb) custom op (bit-NOT recip, deg-1 poly),
          g = max(c+0.5, sigma_c) stt from PSUM, w = 1-f tensor_scalar,
          tensor_tensor_scan (fp32 state, f16 data)
    Pool: v = w * g tensor_tensor
    PE  : 11 DoubleRow matmuls per (chunk, ht)

Sharding: 8 cores, core c -> (sample b = c//2, H-half hh = c%2, 256 ch).
Fully independent cores, no collectives. Host packs x into fp8 pair-layout
streams [128, kp, i, T]; host assembles the output (f16 -> f32).
"""

from contextlib import ExitStack

import numpy as np
import ml_dtypes

import concourse.bacc as bacc
import concourse.tile as tile
import concourse.mybir as mybir
from concourse.bass_utils import run_bass_kernel_spmd

import concourse.dve_ops as _dve_ops
from concourse.dve_spec import (Spec as _Spec, Src0 as _S0, Src1 as _S1,
                                C0 as _C0, C1 as _C1,
                                AluOp as _AluOp, Bin as _Bin, lower as _lower)
from concourse.dve_uop import DveOpSpec as _DveOpSpec
from concourse.dve_table_gen import dve_ver_for as _dve_ver_for

F8 = mybir.dt.float8e4
F16 = mybir.dt.float16
F32 = mybir.dt.float32
AF = mybir.ActivationFunctionType
OP = mybir.AluOpType
PM = mybir.MatmulPerfMode

NPF8 = ml_dtypes.float8_e4m3

# ---- fused custom DVE op: f = in0 / (in0 + in1) ---------------------------
# x = in0+in1 (positive); nx = bitcast(~x) seed; u = x*nx in [-4.5,-4);
# f = ((u*c0 + c1) * nx) * in0.  Deg-1 minimax on u: max rel err 2.9e-3.
FDIV_CONSTS = {"s0": -0.05551854, "s1": -0.47192850}


def _register_fdiv():
    name = "FRAC_SIGMOID_ANT"
    if name in _dve_ops._SUB_OPCODE_FOR_NAME:
        return next(o for o in _dve_ops.OPS if o.name == name)
    _x = _S0 + _S1
    _nx = _Bin(_AluOp.BITWISE_NOT, _x, _x)
    _u = _x * _nx
    _f = ((_u * _C0 + _C1) * _nx) * _S0

    def _ref(in0, in1, s0, s1):
        a = np.asarray(in0, np.float32)
        b = np.asarray(in1, np.float32)
        x = (a + b).astype(np.float32)
        nx = (~x.view(np.int32)).view(np.float32)
        u = (x * nx).astype(np.float32)
        return ((u * s0 + s1) * nx * a).astype(np.float32)

    spec = _Spec(body=_f, reference=_ref)
    row = _dve_ops._CUSTOM_DVE_ROW_BASE + len(_dve_ops.OPS)
    assert row < 0x20
    ver = _dve_ver_for("TRN2")
    sha = _DveOpSpec(name=name, opcode=row, uops=_lower(spec, ver=ver),
                     rd1_en=True).sha(ver)
    op = _dve_ops.DveOp(name, spec, subdim=False, uops_sha={ver: sha})
    _dve_ops.OPS.append(op)
    _dve_ops.CUSTOM_DVE_SPECS[name] = spec
    _dve_ops._SUB_OPCODE_FOR_NAME[name] = row
    return op


FDIV_OP = _register_fdiv()

B, T, D, H = 4, 8192, 512, 512
NCORES = 8
HS = H // 2          # 256 channels per core
NHT = 2              # 128-channel tiles per core
TC = 512             # T chunk width
UC = 2 * TC          # unit = 2 chunks
NU = T // UC         # 8 units
WSCALE = 64.0        # weight scale (keeps W residual out of e4m3 subnormals)

PSCAN_DEFER = 4
MM_MODE = "fp8"      # kept for test.py compatibility

_nc_cache = {}


def _build_nc(mm_mode=MM_MODE):
    nc = bacc.Bacc("TRN2", target_bir_lowering=False, debug=False,
                   num_devices=NCORES)
    # fp8 pair-layout x streams: [p, kp, i, t] ; d = kp*256 + i*128 + p
    x8d = nc.dram_tensor("x8", [128, 2, 2, T], F8, kind="ExternalInput")
    xrd = nc.dram_tensor("xr", [128, 2, 2, T], F8, kind="ExternalInput")
    # stationary: 21 pair-slices [p, sl, i, m]; per ht: wf0 wf1 wi0 wi1
    # whi0 whi1 elo0 elo1 wxr0 wxr1 ; slice 20 = bias 0.125
    wd = nc.dram_tensor("w", [128, 21, 2, 128], F8, kind="ExternalInput")
    onesd = nc.dram_tensor("ones", [128, 2, UC], F8, kind="ExternalInput")
    # aux cols: 0..1 = g(h_0) per ht ; 2 = -0.5 (sigmoid bias)
    auxd = nc.dram_tensor("aux", [128, 3], F32, kind="ExternalInput")
    outd = nc.dram_tensor("out", [128, NHT, T], F16, kind="ExternalOutput")

    with tile.TileContext(nc) as tc, ExitStack() as ctx:
        wpool = ctx.enter_context(tc.tile_pool(name="w", bufs=1))
        xpool = ctx.enter_context(tc.tile_pool(name="x", bufs=2))
        gpool = ctx.enter_context(tc.tile_pool(name="g", bufs=3))
        hpool = ctx.enter_context(tc.tile_pool(name="h", bufs=3))
        ppool = ctx.enter_context(tc.tile_pool(name="p", bufs=1, space="PSUM"))

        wt = wpool.tile([128, 21, 2, 128], F8, tag="w")
        nc.scalar.dma_start(wt[:], wd[:])
        onest = wpool.tile([128, 2, UC], F8, tag="ones")
        nc.scalar.dma_start(onest[:], onesd[:])
        auxt = wpool.tile([128, 3], F32, tag="aux")
        nc.scalar.dma_start(auxt[:], auxd[:])

        carry = [None] * NHT
        pending_out = []
        pending_scan = []
        units = {}

        def _emit_scan(ht, pu):
            pf, pv, ph = units[pu]
            ini = auxt[:, ht:ht + 1] if pu == 0 else carry[ht]
            nc.vector.tensor_tensor_scan(ph[ht][:], pf[ht][:], pv[ht][:], ini,
                                         OP.mult, OP.add)
            carry[ht] = ph[ht][:, UC - 1:UC]

        for u in range(NU):
            usl = slice(u * UC, (u + 1) * UC)
            x8t = xpool.tile([128, 2, 2, UC], F8, tag="x8", name="x8")
            xrt = xpool.tile([128, 2, 2, UC], F8, tag="xr", name="xr")
            if u == 0:
                # split the first loads per chunk so the pipeline fills early
                for k in range(2):
                    kx = slice(k * TC, (k + 1) * TC)
                    nc.sync.dma_start(x8t[:, :, :, kx], x8d[:, :, :, kx])
                    nc.scalar.dma_start(xrt[:, :, :, kx], xrd[:, :, :, kx])
            else:
                nc.sync.dma_start(x8t[:], x8d[:, :, :, usl])
                nc.scalar.dma_start(xrt[:], xrd[:, :, :, usl])

            sabg = [gpool.tile([128, 3, UC], F16, tag=f"s{ht}", name="sabg")
                    for ht in range(NHT)]
            g2 = [gpool.tile([128, UC], F16, tag=f"g{ht}", name="g2")
                  for ht in range(NHT)]
            f2u = [gpool.tile([128, UC], F16, tag=f"f{ht}", name="f2")
                   for ht in range(NHT)]
            w2u = [gpool.tile([128, UC], F16, tag=f"w{ht}", name="w2")
                   for ht in range(NHT)]
            v2u = [gpool.tile([128, UC], F16, tag=f"v{ht}", name="v2")
                   for ht in range(NHT)]
            h2u = [hpool.tile([128, UC], F16, tag=f"h{ht}", name="h2")
                   for ht in range(NHT)]
            units[u] = (f2u, v2u, h2u)
            # unit-wide pc: both chunks of the c-projection live at once so
            # sigma_c and g run once per unit at 1024 cols
            pcu = [ppool.tile([128, 2, TC], F32, tag=f"c{ht}", name=f"c{ht}")
                   for ht in range(NHT)]

            # chunk-major emission: the two ht chains interleave in every
            # in-order sequencer queue, so a stalled chunk of one chain never
            # blocks the ready chunk of the other
            for k in range(2):
                ksl = slice(k * TC, (k + 1) * TC)
                pab = [None] * NHT
                for ht in range(NHT):
                    W0 = ht * 10
                    pab[ht] = ppool.tile([128, 2, TC], F32, tag=f"p{ht}",
                                         name=f"p{ht}")
                    for kp in range(2):  # a, b projections
                        nc.tensor.matmul(pab[ht][:, 0, :],
                                         wt[:, W0 + kp, :, :],
                                         x8t[:, kp, :, ksl], start=(kp == 0),
                                         stop=(kp == 1),
                                         perf_mode=PM.DoubleRow)
                        nc.tensor.matmul(pab[ht][:, 1, :],
                                         wt[:, W0 + 2 + kp, :, :],
                                         x8t[:, kp, :, ksl], start=(kp == 0),
                                         stop=(kp == 1),
                                         perf_mode=PM.DoubleRow)
                    # c+0.5: x8@Whi + x8@Elo + xr@Wxr (Wxr row 511 = +32)
                    for j, (wsl, xt) in enumerate([(W0 + 4, x8t),
                                                   (W0 + 6, x8t),
                                                   (W0 + 8, xrt)]):
                        for kp in range(2):
                            nc.tensor.matmul(pcu[ht][:, k, :],
                                             wt[:, wsl + kp, :, :],
                                             xt[:, kp, :, ksl],
                                             start=(j == 0 and kp == 0),
                                             stop=(j == 2 and kp == 1),
                                             perf_mode=PM.DoubleRow)
                for ht in range(NHT):
                    nc.scalar.activation(sabg[ht][:, 0:2, ksl], pab[ht][:],
                                         AF.Sigmoid, scale=1.0 / 64.0)
            for ht in range(NHT):
                # sigma_c = sigmoid(pc/64 - 0.5) = sigmoid(c), whole unit
                sgc = sabg[ht][:, 2, :].rearrange("p (a b) -> p a b", a=2)
                nc.scalar.activation(sgc, pcu[ht][:], AF.Sigmoid,
                                     bias=auxt[:, 2:3], scale=1.0 / 64.0)
            for ht in range(NHT):
                # per-ht tail emitted as [g, A, w, v] so each ht's v reaches
                # Pool right after its own w, mid-block, with the other ht's
                # DVE work left to overlap the Pool multiply
                g3 = g2[ht][:].rearrange("p (a b) -> p a b", a=2)
                sg3 = sabg[ht][:, 2, :].rearrange("p (a b) -> p a b", a=2)
                # g = max(c + 0.5, sigma_c)   (PSUM read -> DVE)
                nc.vector.scalar_tensor_tensor(g3, pcu[ht][:], 1.0 / 64.0,
                                               sg3, OP.mult, OP.max)
                nc.vector._custom_dve(FDIV_OP, out=f2u[ht][:],
                                      in0=sabg[ht][:, 0, :],
                                      in1=sabg[ht][:, 1, :],
                                      s0=FDIV_CONSTS["s0"],
                                      s1=FDIV_CONSTS["s1"])
                nc.vector.tensor_scalar(w2u[ht][:], f2u[ht][:], -1.0, 1.0,
                                        OP.mult, OP.add)
                if ht == 0:
                    nc.gpsimd.tensor_tensor(v2u[ht][:], w2u[ht][:], g2[ht][:],
                                            op=OP.mult)
                else:
                    nc.vector.tensor_tensor(v2u[ht][:], w2u[ht][:], g2[ht][:],
                                            op=OP.mult)
                # scans are emitted one unit late so a scan waiting on
                # Pool's v never head-blocks the DVE queue
                pending_scan.append((ht, u))
            while len(pending_scan) > NHT:
                ph, pu = pending_scan.pop(0)
                _emit_scan(ph, pu)
            for ht in range(NHT):
                pending_out.append((ht, usl, h2u[ht]))
            # emit the previous unit's output DMAs here (one unit late, on
            # the ACT queue) so their scan-chain waits never sit in front of
            # the x prefetches or the sigma dispatches in a sequencer queue
            while len(pending_out) > NHT:
                oht, ousl, oh2 = pending_out.pop(0)
                nc.scalar.dma_start(outd[:, oht, ousl], oh2[:])
        while pending_scan:
            ph, pu = pending_scan.pop(0)
            _emit_scan(ph, pu)
        # final unit: split the store per chunk so the drain overlaps
        while pending_out:
            oht, ousl, oh2 = pending_out.pop(0)
            for k in range(2):
                osl = slice(ousl.start + k * TC, ousl.start + (k + 1) * TC)
                nc.scalar.dma_start(outd[:, oht, osl], oh2[:, k * TC:(k + 1) * TC])
    nc.compile()
    return nc


def _get_nc(mm_mode=MM_MODE):
    if mm_mode not in _nc_cache:
        _nc_cache[mm_mode] = _build_nc(mm_mode)
    return _nc_cache[mm_mode]


def _g_host(x):
    # exp(log_g(x)) of the reference, computed directly in fp32
    return np.where(x >= 0, x + 0.5, 1.0 / (1.0 + np.exp(-np.minimum(x, 0))))


def _q8(a):
    return a.astype(NPF8)


def _pack_x(xT_f32):
    """[D, T] f32 -> (x8 pack, xr pack) in [128, kp, i, T] fp8 pair layout;
    xr row d=511 is the constant 1.0 that delivers the c-gate's +32 bias."""
    x8 = _q8(xT_f32)
    xr = _q8(xT_f32 - x8.astype(np.float32))
    xr[511, :] = NPF8(1.0)
    def pack(a):
        return np.ascontiguousarray(
            a.reshape(2, 2, 128, T).transpose(2, 0, 1, 3))
    return pack(x8), pack(xr)


def _pack_w_slices(mat, ht):
    """[512, 256] fp8 -> two [128, 2, 128] pair slices (kp = 0, 1)."""
    r = mat.reshape(2, 2, 128, 2, 128)  # [kp, i, p, ht, m]
    return [np.ascontiguousarray(r[kp, :, :, ht, :].transpose(1, 0, 2))
            for kp in range(2)]


def _run(inputs, mm_mode=MM_MODE, trace=False):
    x = np.asarray(inputs["x"], np.float32)
    h_0 = np.asarray(inputs["h_0"], np.float32)
    W_f = np.asarray(inputs["W_f"], np.float32)
    b_f = np.asarray(inputs["b_f"], np.float32)
    W_i = np.asarray(inputs["W_i"], np.float32)
    b_i = np.asarray(inputs["b_i"], np.float32)
    W_h = np.asarray(inputs["W_h"], np.float32)
    b_h = np.asarray(inputs["b_h"], np.float32)
    assert (b_f == 0).all() and (b_i == 0).all() and (b_h == 0).all(), \
        "device program folds zero biases"

    g0 = _g_host(h_0[:, 0, :])  # [B, H]
    xpacks = [_pack_x(np.ascontiguousarray(x[b].T)) for b in range(B)]

    ones = np.ones((128, 2, UC), NPF8)
    in_maps = []
    for c in range(NCORES):
        b, hh = divmod(c, 2)
        hs = slice(hh * HS, (hh + 1) * HS)
        wf8 = _q8(WSCALE * W_f[:, hs])
        wi8 = _q8(WSCALE * W_i[:, hs])
        whi = _q8(WSCALE * W_h[:, hs])
        elo = _q8(WSCALE * W_h[:, hs] - whi.astype(np.float32))
        wxr = whi.copy()
        wxr[511, :] = NPF8(32.0)
        wcat = np.zeros((128, 21, 2, 128), NPF8)
        for ht in range(NHT):
            for mi, mat in enumerate((wf8, wi8, whi, elo, wxr)):
                s0, s1 = _pack_w_slices(mat, ht)
                wcat[:, ht * 10 + mi * 2, :, :] = s0
                wcat[:, ht * 10 + mi * 2 + 1, :, :] = s1
        wcat[:, 20, :, :] = NPF8(0.125)
        aux = np.empty((128, 3), np.float32)
        aux[:, 0:2] = g0[b, hs].reshape(2, 128).T
        aux[:, 2] = -0.5
        x8p, xrp = xpacks[b]
        in_maps.append({"x8": x8p, "xr": xrp, "w": wcat, "ones": ones,
                        "aux": aux})

    nc = _get_nc(mm_mode)
    res = run_bass_kernel_spmd(nc, in_maps, core_ids=list(range(NCORES)),
                               trace=trace)

    out = np.empty((B, T + 1, H), np.float32)
    out[:, 0, :] = g0
    for c in range(NCORES):
        b, hh = divmod(c, 2)
        hs = slice(hh * HS, (hh + 1) * HS)
        r = np.asarray(res.results[c]["out"], np.float32)  # [128, NHT, T]
        out[b, 1:, hs] = r.transpose(2, 1, 0).reshape(T, HS)
    return out, res


def kernel(**inputs):
    out, _ = _run(inputs)
    return out


# revision 34
# speedup vs baseline: 1.1179x; 1.0445x over previous
"""MinLSTM Trainium2 kernel (fp8-DoubleRow edition).

Full-input contract: kernel(**inputs) takes the complete (unsharded) numpy
inputs of the reference model and returns the full [B, T+1, H] float32 output.

Math (identical to the reference's log-space scan, computed in linear space):
    a = x @ W_f ;  b = x @ W_i ;  c = x @ W_h       (zero biases asserted)
    f = sigmoid(a) / (sigmoid(a) + sigmoid(b))      # forget gate
    i = 1 - f                                       # input gate
    g = max(c + 0.5, sigmoid(c))                    # = exp(log_g(c))
    h_t = f_t * h_{t-1} + i_t * g_t,   h_{-1} = g(h_0)

Matmul scheme (all fp8 e4m3 with DoubleRow perf mode, 2 k-tiles/instr at
0.5 cycles/row = 4x the f32r rate). Weights are scaled by 64 so the W
residual of the h-projection is representable in e4m3; every PSUM slice
holds 64*(pre + 0.5) and one sigmoid pass applies scale=1/64, bias=-0.5:
    a_psum = ones*0.125(bias=+32) + x8 @ q8(64 W_f)            (plain)
    b_psum = ones*0.125(bias=+32) + x8 @ q8(64 W_i)            (plain)
    c_psum = x8 @ Whi + x8 @ Elo + xr @ Wxr                    (corrected)
        Whi = q8(64 W_h); Elo = q8(64 W_h - Whi)               (W residual)
        xr  = q8(x - x8) with row 511 := 1.0                   (x residual)
        Wxr = Whi with row 511 := 32.0                         (c's +32 bias)
The c-projection carries both residual corrections because h is ~1:1
sensitive to g but only ~0.3x to f (measured): end-to-end l2 ~ 6.5e-3.

Per-core engine placement (GPSIMD can't touch PSUM or TensorScalarPtr ops):
    ACT : one sigmoid pass per (chunk, ht) over [128, 3, TC] PSUM
    DVE : fused f = sa/(sa+sb) custom op (bit-NOT recip, deg-1 poly),
          g = max(c+0.5, sigma_c) stt from PSUM, w = 1-f tensor_scalar,
          tensor_tensor_scan (fp32 state, f16 data)
    Pool: v = w * g tensor_tensor
    PE  : 11 DoubleRow matmuls per (chunk, ht)

Sharding: 8 cores, core c -> (sample b = c//2, H-half hh = c%2, 256 ch).
Fully independent cores, no collectives. Host packs x into fp8 pair-layout
streams [128, kp, i, T]; host assembles the output (f16 -> f32).
"""

from contextlib import ExitStack

import numpy as np
import ml_dtypes

import concourse.bacc as bacc
import concourse.tile as tile
import concourse.mybir as mybir
from concourse.bass_utils import run_bass_kernel_spmd

import concourse.dve_ops as _dve_ops
from concourse.dve_spec import (Spec as _Spec, Src0 as _S0, Src1 as _S1,
                                C0 as _C0, C1 as _C1,
                                AluOp as _AluOp, Bin as _Bin, lower as _lower)
from concourse.dve_uop import DveOpSpec as _DveOpSpec
from concourse.dve_table_gen import dve_ver_for as _dve_ver_for

F8 = mybir.dt.float8e4
F16 = mybir.dt.float16
F32 = mybir.dt.float32
AF = mybir.ActivationFunctionType
OP = mybir.AluOpType
PM = mybir.MatmulPerfMode

NPF8 = ml_dtypes.float8_e4m3

# ---- fused custom DVE op: f = in0 / (in0 + in1) ---------------------------
# x = in0+in1 (positive); nx = bitcast(~x) seed; u = x*nx in [-4.5,-4);
# f = ((u*c0 + c1) * nx) * in0.  Deg-1 minimax on u: max rel err 2.9e-3.
FDIV_CONSTS = {"s0": -0.05551854, "s1": -0.47192850}


def _register_fdiv():
    name = "FRAC_SIGMOID_ANT"
    if name in _dve_ops._SUB_OPCODE_FOR_NAME:
        return next(o for o in _dve_ops.OPS if o.name == name)
    _x = _S0 + _S1
    _nx = _Bin(_AluOp.BITWISE_NOT, _x, _x)
    _u = _x * _nx
    _f = ((_u * _C0 + _C1) * _nx) * _S0

    def _ref(in0, in1, s0, s1):
        a = np.asarray(in0, np.float32)
        b = np.asarray(in1, np.float32)
        x = (a + b).astype(np.float32)
        nx = (~x.view(np.int32)).view(np.float32)
        u = (x * nx).astype(np.float32)
        return ((u * s0 + s1) * nx * a).astype(np.float32)

    spec = _Spec(body=_f, reference=_ref)
    row = _dve_ops._CUSTOM_DVE_ROW_BASE + len(_dve_ops.OPS)
    assert row < 0x20
    ver = _dve_ver_for("TRN2")
    sha = _DveOpSpec(name=name, opcode=row, uops=_lower(spec, ver=ver),
                     rd1_en=True).sha(ver)
    op = _dve_ops.DveOp(name, spec, subdim=False, uops_sha={ver: sha})
    _dve_ops.OPS.append(op)
    _dve_ops.CUSTOM_DVE_SPECS[name] = spec
    _dve_ops._SUB_OPCODE_FOR_NAME[name] = row
    return op


FDIV_OP = _register_fdiv()

B, T, D, H = 4, 8192, 512, 512
NCORES = 8
HS = H // 2          # 256 channels per core
NHT = 2              # 128-channel tiles per core
TC = 512             # T chunk width
UC = 2 * TC          # unit = 2 chunks
NU = T // UC         # 8 units
WSCALE = 64.0        # weight scale (keeps W residual out of e4m3 subnormals)

PSCAN_DEFER = 4
MM_MODE = "fp8"      # kept for test.py compatibility

_nc_cache = {}


def _build_nc(mm_mode=MM_MODE):
    nc = bacc.Bacc("TRN2", target_bir_lowering=False, debug=False,
                   num_devices=NCORES)
    # fp8 pair-layout x streams: [p, kp, i, t] ; d = kp*256 + i*128 + p
    x8d = nc.dram_tensor("x8", [128, 2, 2, T], F8, kind="ExternalInput")
    xrd = nc.dram_tensor("xr", [128, 2, 2, T], F8, kind="ExternalInput")
    # stationary: 21 pair-slices [p, sl, i, m]; per ht: wf0 wf1 wi0 wi1
    # whi0 whi1 elo0 elo1 wxr0 wxr1 ; slice 20 = bias 0.125
    wd = nc.dram_tensor("w", [128, 21, 2, 128], F8, kind="ExternalInput")
    onesd = nc.dram_tensor("ones", [128, 2, UC], F8, kind="ExternalInput")
    # aux cols: 0..1 = g(h_0) per ht ; 2 = -0.5 (sigmoid bias)
    auxd = nc.dram_tensor("aux", [128, 3], F32, kind="ExternalInput")
    outd = nc.dram_tensor("out", [128, NHT, T], F16, kind="ExternalOutput")

    with tile.TileContext(nc) as tc, ExitStack() as ctx:
        wpool = ctx.enter_context(tc.tile_pool(name="w", bufs=1))
        xpool = ctx.enter_context(tc.tile_pool(name="x", bufs=2))
        gpool = ctx.enter_context(tc.tile_pool(name="g", bufs=3))
        hpool = ctx.enter_context(tc.tile_pool(name="h", bufs=3))
        ppool = ctx.enter_context(tc.tile_pool(name="p", bufs=1, space="PSUM"))

        wt = wpool.tile([128, 21, 2, 128], F8, tag="w")
        nc.scalar.dma_start(wt[:], wd[:])
        onest = wpool.tile([128, 2, UC], F8, tag="ones")
        nc.scalar.dma_start(onest[:], onesd[:])
        auxt = wpool.tile([128, 3], F32, tag="aux")
        nc.scalar.dma_start(auxt[:], auxd[:])

        carry = [None] * NHT
        pending_out = []
        pending_scan = []
        units = {}

        def _emit_scan(ht, pu):
            pf, pv, ph = units[pu]
            ini = auxt[:, ht:ht + 1] if pu == 0 else carry[ht]
            nc.vector.tensor_tensor_scan(ph[ht][:], pf[ht][:], pv[ht][:], ini,
                                         OP.mult, OP.add)
            carry[ht] = ph[ht][:, UC - 1:UC]

        for u in range(NU):
            usl = slice(u * UC, (u + 1) * UC)
            x8t = xpool.tile([128, 2, 2, UC], F8, tag="x8", name="x8")
            xrt = xpool.tile([128, 2, 2, UC], F8, tag="xr", name="xr")
            if u == 0:
                # split the first loads per chunk so the pipeline fills early
                for k in range(2):
                    kx = slice(k * TC, (k + 1) * TC)
                    nc.sync.dma_start(x8t[:, :, :, kx], x8d[:, :, :, kx])
                    nc.scalar.dma_start(xrt[:, :, :, kx], xrd[:, :, :, kx])
            else:
                nc.sync.dma_start(x8t[:], x8d[:, :, :, usl])
                nc.scalar.dma_start(xrt[:], xrd[:, :, :, usl])

            sabg = [gpool.tile([128, 3, UC], F16, tag=f"s{ht}", name="sabg")
                    for ht in range(NHT)]
            g2 = [gpool.tile([128, UC], F16, tag=f"g{ht}", name="g2")
                  for ht in range(NHT)]
            f2u = [gpool.tile([128, UC], F16, tag=f"f{ht}", name="f2")
                   for ht in range(NHT)]
            w2u = [gpool.tile([128, UC], F16, tag=f"w{ht}", name="w2")
                   for ht in range(NHT)]
            v2u = [gpool.tile([128, UC], F16, tag=f"v{ht}", name="v2")
                   for ht in range(NHT)]
            h2u = [hpool.tile([128, UC], F16, tag=f"h{ht}", name="h2")
                   for ht in range(NHT)]
            units[u] = (f2u, v2u, h2u)
            # unit-wide pc: both chunks of the c-projection live at once so
            # sigma_c and g run once per unit at 1024 cols
            pcu = [ppool.tile([128, 2, TC], F32, tag=f"c{ht}", name=f"c{ht}")
                   for ht in range(NHT)]

            # chunk-major emission: the two ht chains interleave in every
            # in-order sequencer queue, so a stalled chunk of one chain never
            # blocks the ready chunk of the other
            for k in range(2):
                ksl = slice(k * TC, (k + 1) * TC)
                pab = [None] * NHT
                for ht in range(NHT):
                    W0 = ht * 10
                    pab[ht] = ppool.tile([128, 2, TC], F32, tag=f"p{ht}",
                                         name=f"p{ht}")
                    for kp in range(2):  # a, b projections
                        nc.tensor.matmul(pab[ht][:, 0, :],
                                         wt[:, W0 + kp, :, :],
                                         x8t[:, kp, :, ksl], start=(kp == 0),
                                         stop=(kp == 1),
                                         perf_mode=PM.DoubleRow)
                        nc.tensor.matmul(pab[ht][:, 1, :],
                                         wt[:, W0 + 2 + kp, :, :],
                                         x8t[:, kp, :, ksl], start=(kp == 0),
                                         stop=(kp == 1),
                                         perf_mode=PM.DoubleRow)
                    # c+0.5: x8@Whi + x8@Elo + xr@Wxr (Wxr row 511 = +32)
                    for j, (wsl, xt) in enumerate([(W0 + 4, x8t),
                                                   (W0 + 6, x8t),
                                                   (W0 + 8, xrt)]):
                        for kp in range(2):
                            nc.tensor.matmul(pcu[ht][:, k, :],
                                             wt[:, wsl + kp, :, :],
                                             xt[:, kp, :, ksl],
                                             start=(j == 0 and kp == 0),
                                             stop=(j == 2 and kp == 1),
                                             perf_mode=PM.DoubleRow)
                for ht in range(NHT):
                    nc.scalar.activation(sabg[ht][:, 0:2, ksl], pab[ht][:],
                                         AF.Sigmoid, scale=1.0 / 64.0)
                if u == 0:
                    # pipeline-fill: run the whole tail per chunk for the
                    # first unit so DVE work starts as early as possible
                    for ht in range(NHT):
                        nc.scalar.activation(sabg[ht][:, 2, ksl],
                                             pcu[ht][:, k, :], AF.Sigmoid,
                                             bias=auxt[:, 2:3],
                                             scale=1.0 / 64.0)
                        nc.vector.scalar_tensor_tensor(
                            g2[ht][:, ksl], pcu[ht][:, k, :], 1.0 / 64.0,
                            sabg[ht][:, 2, ksl], OP.mult, OP.max)
                        nc.vector._custom_dve(FDIV_OP, out=f2u[ht][:, ksl],
                                              in0=sabg[ht][:, 0, ksl],
                                              in1=sabg[ht][:, 1, ksl],
                                              s0=FDIV_CONSTS["s0"],
                                              s1=FDIV_CONSTS["s1"])
                        nc.vector.tensor_scalar(w2u[ht][:, ksl],
                                                f2u[ht][:, ksl], -1.0, 1.0,
                                                OP.mult, OP.add)
                        nc.gpsimd.tensor_tensor(v2u[ht][:, ksl],
                                                w2u[ht][:, ksl],
                                                g2[ht][:, ksl], op=OP.mult)
            if u == 0:
                for ht in range(NHT):
                    pending_scan.append((ht, u))
                for ht in range(NHT):
                    pending_out.append((ht, usl, h2u[ht]))
                continue
            for ht in range(NHT):
                # sigma_c = sigmoid(pc/64 - 0.5) = sigmoid(c), whole unit
                sgc = sabg[ht][:, 2, :].rearrange("p (a b) -> p a b", a=2)
                nc.scalar.activation(sgc, pcu[ht][:], AF.Sigmoid,
                                     bias=auxt[:, 2:3], scale=1.0 / 64.0)
            for ht in range(NHT):
                # per-ht tail emitted as [g, A, w, v] so each ht's v reaches
                # Pool right after its own w, mid-block, with the other ht's
                # DVE work left to overlap the Pool multiply
                g3 = g2[ht][:].rearrange("p (a b) -> p a b", a=2)
                sg3 = sabg[ht][:, 2, :].rearrange("p (a b) -> p a b", a=2)
                # g = max(c + 0.5, sigma_c)   (PSUM read -> DVE)
                nc.vector.scalar_tensor_tensor(g3, pcu[ht][:], 1.0 / 64.0,
                                               sg3, OP.mult, OP.max)
                nc.vector._custom_dve(FDIV_OP, out=f2u[ht][:],
                                      in0=sabg[ht][:, 0, :],
                                      in1=sabg[ht][:, 1, :],
                                      s0=FDIV_CONSTS["s0"],
                                      s1=FDIV_CONSTS["s1"])
                nc.vector.tensor_scalar(w2u[ht][:], f2u[ht][:], -1.0, 1.0,
                                        OP.mult, OP.add)
                if ht == 0:
                    nc.gpsimd.tensor_tensor(v2u[ht][:], w2u[ht][:], g2[ht][:],
                                            op=OP.mult)
                else:
                    # split: first half to Pool (fits before the deferred
                    # scan), second half on DVE
                    nc.gpsimd.tensor_tensor(v2u[ht][:, :TC], w2u[ht][:, :TC],
                                            g2[ht][:, :TC], op=OP.mult)
                    nc.vector.tensor_tensor(v2u[ht][:, TC:], w2u[ht][:, TC:],
                                            g2[ht][:, TC:], op=OP.mult)
                # scans are emitted one unit late so a scan waiting on
                # Pool's v never head-blocks the DVE queue
                pending_scan.append((ht, u))
            while len(pending_scan) > NHT:
                ph, pu = pending_scan.pop(0)
                _emit_scan(ph, pu)
            for ht in range(NHT):
                pending_out.append((ht, usl, h2u[ht]))
            # emit the previous unit's output DMAs here (one unit late, on
            # the ACT queue) so their scan-chain waits never sit in front of
            # the x prefetches or the sigma dispatches in a sequencer queue
            while len(pending_out) > NHT:
                oht, ousl, oh2 = pending_out.pop(0)
                nc.scalar.dma_start(outd[:, oht, ousl], oh2[:])
        while pending_scan:
            ph, pu = pending_scan.pop(0)
            pf, pv, phh = units[pu]
            for k in range(2):
                kq = slice(k * TC, (k + 1) * TC)
                ini = (auxt[:, ph:ph + 1] if (pu == 0 and k == 0)
                       else carry[ph])
                nc.vector.tensor_tensor_scan(phh[ph][:, kq], pf[ph][:, kq],
                                             pv[ph][:, kq], ini,
                                             OP.mult, OP.add)
                carry[ph] = phh[ph][:, (k + 1) * TC - 1:(k + 1) * TC]
        # final unit: split the store per chunk so the drain overlaps
        while pending_out:
            oht, ousl, oh2 = pending_out.pop(0)
            for k in range(2):
                osl = slice(ousl.start + k * TC, ousl.start + (k + 1) * TC)
                nc.scalar.dma_start(outd[:, oht, osl], oh2[:, k * TC:(k + 1) * TC])
    nc.compile()
    return nc


def _get_nc(mm_mode=MM_MODE):
    if mm_mode not in _nc_cache:
        _nc_cache[mm_mode] = _build_nc(mm_mode)
    return _nc_cache[mm_mode]


def _g_host(x):
    # exp(log_g(x)) of the reference, computed directly in fp32
    return np.where(x >= 0, x + 0.5, 1.0 / (1.0 + np.exp(-np.minimum(x, 0))))


def _q8(a):
    return a.astype(NPF8)


def _pack_x(xT_f32):
    """[D, T] f32 -> (x8 pack, xr pack) in [128, kp, i, T] fp8 pair layout;
    xr row d=511 is the constant 1.0 that delivers the c-gate's +32 bias."""
    x8 = _q8(xT_f32)
    xr = _q8(xT_f32 - x8.astype(np.float32))
    xr[511, :] = NPF8(1.0)
    def pack(a):
        return np.ascontiguousarray(
            a.reshape(2, 2, 128, T).transpose(2, 0, 1, 3))
    return pack(x8), pack(xr)


def _pack_w_slices(mat, ht):
    """[512, 256] fp8 -> two [128, 2, 128] pair slices (kp = 0, 1)."""
    r = mat.reshape(2, 2, 128, 2, 128)  # [kp, i, p, ht, m]
    return [np.ascontiguousarray(r[kp, :, :, ht, :].transpose(1, 0, 2))
            for kp in range(2)]


def _run(inputs, mm_mode=MM_MODE, trace=False):
    x = np.asarray(inputs["x"], np.float32)
    h_0 = np.asarray(inputs["h_0"], np.float32)
    W_f = np.asarray(inputs["W_f"], np.float32)
    b_f = np.asarray(inputs["b_f"], np.float32)
    W_i = np.asarray(inputs["W_i"], np.float32)
    b_i = np.asarray(inputs["b_i"], np.float32)
    W_h = np.asarray(inputs["W_h"], np.float32)
    b_h = np.asarray(inputs["b_h"], np.float32)
    assert (b_f == 0).all() and (b_i == 0).all() and (b_h == 0).all(), \
        "device program folds zero biases"

    g0 = _g_host(h_0[:, 0, :])  # [B, H]
    xpacks = [_pack_x(np.ascontiguousarray(x[b].T)) for b in range(B)]

    ones = np.ones((128, 2, UC), NPF8)
    in_maps = []
    for c in range(NCORES):
        b, hh = divmod(c, 2)
        hs = slice(hh * HS, (hh + 1) * HS)
        wf8 = _q8(WSCALE * W_f[:, hs])
        wi8 = _q8(WSCALE * W_i[:, hs])
        whi = _q8(WSCALE * W_h[:, hs])
        elo = _q8(WSCALE * W_h[:, hs] - whi.astype(np.float32))
        wxr = whi.copy()
        wxr[511, :] = NPF8(32.0)
        wcat = np.zeros((128, 21, 2, 128), NPF8)
        for ht in range(NHT):
            for mi, mat in enumerate((wf8, wi8, whi, elo, wxr)):
                s0, s1 = _pack_w_slices(mat, ht)
                wcat[:, ht * 10 + mi * 2, :, :] = s0
                wcat[:, ht * 10 + mi * 2 + 1, :, :] = s1
        wcat[:, 20, :, :] = NPF8(0.125)
        aux = np.empty((128, 3), np.float32)
        aux[:, 0:2] = g0[b, hs].reshape(2, 128).T
        aux[:, 2] = -0.5
        x8p, xrp = xpacks[b]
        in_maps.append({"x8": x8p, "xr": xrp, "w": wcat, "ones": ones,
                        "aux": aux})

    nc = _get_nc(mm_mode)
    res = run_bass_kernel_spmd(nc, in_maps, core_ids=list(range(NCORES)),
                               trace=trace)

    out = np.empty((B, T + 1, H), np.float32)
    out[:, 0, :] = g0
    for c in range(NCORES):
        b, hh = divmod(c, 2)
        hs = slice(hh * HS, (hh + 1) * HS)
        r = np.asarray(res.results[c]["out"], np.float32)  # [128, NHT, T]
        out[b, 1:, hs] = r.transpose(2, 1, 0).reshape(T, HS)
    return out, res


def kernel(**inputs):
    out, _ = _run(inputs)
    return out


# revision 43
# speedup vs baseline: 1.1452x; 1.0245x over previous
"""MinLSTM Trainium2 kernel (fp8-DoubleRow edition).

Full-input contract: kernel(**inputs) takes the complete (unsharded) numpy
inputs of the reference model and returns the full [B, T+1, H] float32 output.

Math (identical to the reference's log-space scan, computed in linear space):
    a = x @ W_f ;  b = x @ W_i ;  c = x @ W_h       (zero biases asserted)
    f = sigmoid(a) / (sigmoid(a) + sigmoid(b))      # forget gate
    i = 1 - f                                       # input gate
    g = max(c + 0.5, sigmoid(c))                    # = exp(log_g(c))
    h_t = f_t * h_{t-1} + i_t * g_t,   h_{-1} = g(h_0)

Matmul scheme (all fp8 e4m3 with DoubleRow perf mode, 2 k-tiles/instr at
0.5 cycles/row = 4x the f32r rate). Weights are scaled by 64 so the W
residual of the h-projection is representable in e4m3; every PSUM slice
holds 64*(pre + 0.5) and one sigmoid pass applies scale=1/64, bias=-0.5:
    a_psum = ones*0.125(bias=+32) + x8 @ q8(64 W_f)            (plain)
    b_psum = ones*0.125(bias=+32) + x8 @ q8(64 W_i)            (plain)
    c_psum = x8 @ Whi + x8 @ Elo + xr @ Wxr                    (corrected)
        Whi = q8(64 W_h); Elo = q8(64 W_h - Whi)               (W residual)
        xr  = q8(x - x8) with row 511 := 1.0                   (x residual)
        Wxr = Whi with row 511 := 32.0                         (c's +32 bias)
The c-projection carries both residual corrections because h is ~1:1
sensitive to g but only ~0.3x to f (measured): end-to-end l2 ~ 6.5e-3.

Per-core engine placement (GPSIMD can't touch PSUM or TensorScalarPtr ops):
    ACT : one sigmoid pass per (chunk, ht) over [128, 3, TC] PSUM
    DVE : fused f = sa/(sa+sb) custom op (bit-NOT recip, deg-1 poly),
          g = max(c+0.5, sigma_c) stt from PSUM, w = 1-f tensor_scalar,
          tensor_tensor_scan (fp32 state, f16 data)
    Pool: v = w * g tensor_tensor
    PE  : 11 DoubleRow matmuls per (chunk, ht)

Sharding: 8 cores, core c -> (sample b = c//2, H-half hh = c%2, 256 ch).
Fully independent cores, no collectives. Host packs x into fp8 pair-layout
streams [128, kp, i, T]; host assembles the output (f16 -> f32).
"""

from contextlib import ExitStack

import numpy as np
import ml_dtypes

import concourse.bacc as bacc
import concourse.tile as tile
import concourse.mybir as mybir
from concourse.bass_utils import run_bass_kernel_spmd

import concourse.dve_ops as _dve_ops
from concourse.dve_spec import (Spec as _Spec, Src0 as _S0, Src1 as _S1,
                                C0 as _C0, C1 as _C1,
                                AluOp as _AluOp, Bin as _Bin, lower as _lower)
from concourse.dve_uop import DveOpSpec as _DveOpSpec
from concourse.dve_table_gen import dve_ver_for as _dve_ver_for

F8 = mybir.dt.float8e4
F16 = mybir.dt.float16
F32 = mybir.dt.float32
AF = mybir.ActivationFunctionType
OP = mybir.AluOpType
PM = mybir.MatmulPerfMode

NPF8 = ml_dtypes.float8_e4m3

# ---- fused custom DVE op: f = in0 / (in0 + in1) ---------------------------
# x = in0+in1 (positive); nx = bitcast(~x) seed; u = x*nx in [-4.5,-4);
# f = ((u*c0 + c1) * nx) * in0.  Deg-1 minimax on u: max rel err 2.9e-3.
FDIV_CONSTS = {"s0": -0.05551854, "s1": -0.47192850}


def _register_fdiv():
    name = "FRAC_SIGMOID_ANT"
    if name in _dve_ops._SUB_OPCODE_FOR_NAME:
        return next(o for o in _dve_ops.OPS if o.name == name)
    _x = _S0 + _S1
    _nx = _Bin(_AluOp.BITWISE_NOT, _x, _x)
    _u = _x * _nx
    _f = ((_u * _C0 + _C1) * _nx) * _S0

    def _ref(in0, in1, s0, s1):
        a = np.asarray(in0, np.float32)
        b = np.asarray(in1, np.float32)
        x = (a + b).astype(np.float32)
        nx = (~x.view(np.int32)).view(np.float32)
        u = (x * nx).astype(np.float32)
        return ((u * s0 + s1) * nx * a).astype(np.float32)

    spec = _Spec(body=_f, reference=_ref)
    row = _dve_ops._CUSTOM_DVE_ROW_BASE + len(_dve_ops.OPS)
    assert row < 0x20
    ver = _dve_ver_for("TRN2")
    sha = _DveOpSpec(name=name, opcode=row, uops=_lower(spec, ver=ver),
                     rd1_en=True).sha(ver)
    op = _dve_ops.DveOp(name, spec, subdim=False, uops_sha={ver: sha})
    _dve_ops.OPS.append(op)
    _dve_ops.CUSTOM_DVE_SPECS[name] = spec
    _dve_ops._SUB_OPCODE_FOR_NAME[name] = row
    return op


FDIV_OP = _register_fdiv()

B, T, D, H = 4, 8192, 512, 512
NCORES = 8
HS = H // 2          # 256 channels per core
NHT = 2              # 128-channel tiles per core
TC = 512             # T chunk width
UC = 2 * TC          # unit = 2 chunks
NU = T // UC         # 8 units
WSCALE = 64.0        # weight scale (keeps W residual out of e4m3 subnormals)

PSCAN_DEFER = 4
MM_MODE = "fp8"      # kept for test.py compatibility

_nc_cache = {}


def _build_nc(mm_mode=MM_MODE):
    nc = bacc.Bacc("TRN2", target_bir_lowering=False, debug=False,
                   num_devices=NCORES)
    # fp8 pair-layout x streams: [p, kp, i, t] ; d = kp*256 + i*128 + p
    x8d = nc.dram_tensor("x8", [128, 2, 2, T], F8, kind="ExternalInput")
    xrd = nc.dram_tensor("xr", [128, 2, 2, T], F8, kind="ExternalInput")
    # stationary: 21 pair-slices [p, sl, i, m]; per ht: wf0 wf1 wi0 wi1
    # whi0 whi1 elo0 elo1 wxr0 wxr1 ; slice 20 = bias 0.125
    wd = nc.dram_tensor("w", [128, 21, 2, 128], F8, kind="ExternalInput")
    # aux cols: 0..1 = g(h_0) per ht ; 2 = -0.5 (sigmoid bias)
    auxd = nc.dram_tensor("aux", [128, 3], F32, kind="ExternalInput")
    outd = nc.dram_tensor("out", [128, NHT, T], F16, kind="ExternalOutput")

    with tile.TileContext(nc) as tc, ExitStack() as ctx:
        wpool = ctx.enter_context(tc.tile_pool(name="w", bufs=1))
        xpool = ctx.enter_context(tc.tile_pool(name="x", bufs=2))
        gpool = ctx.enter_context(tc.tile_pool(name="g", bufs=3))
        hpool = ctx.enter_context(tc.tile_pool(name="h", bufs=3))
        ppool = ctx.enter_context(tc.tile_pool(name="p", bufs=1, space="PSUM"))

        wt = wpool.tile([128, 21, 2, 128], F8, tag="w")
        auxt = wpool.tile([128, 3], F32, tag="aux")
        nc.scalar.dma_start(wt[:, 0:10, :, :], wd[:, 0:10, :, :])
        nc.scalar.dma_start(wt[:, 10:21, :, :], wd[:, 10:21, :, :])
        nc.scalar.dma_start(auxt[:], auxd[:])

        carry = [None] * NHT
        pending_out = []
        pending_scan = []
        units = {}

        def _emit_scan(ht, pu):
            pf, pv, ph = units[pu]
            ini = auxt[:, ht:ht + 1] if pu == 0 else carry[ht]
            nc.vector.tensor_tensor_scan(ph[ht][:], pf[ht][:], pv[ht][:], ini,
                                         OP.mult, OP.add)
            carry[ht] = ph[ht][:, UC - 1:UC]

        for u in range(NU):
            usl = slice(u * UC, (u + 1) * UC)
            x8t = xpool.tile([128, 2, 2, UC], F8, tag="x8", name="x8")
            xrt = xpool.tile([128, 2, 2, UC], F8, tag="xr", name="xr")
            if u == 0:
                # split the first loads per chunk so the pipeline fills early
                for k in range(2):
                    kx = slice(k * TC, (k + 1) * TC)
                    nc.sync.dma_start(x8t[:, :, :, kx], x8d[:, :, :, kx])
                    nc.scalar.dma_start(xrt[:, :, :, kx], xrd[:, :, :, kx])
            else:
                nc.sync.dma_start(x8t[:], x8d[:, :, :, usl])
                nc.scalar.dma_start(xrt[:], xrd[:, :, :, usl])

            sabg = [gpool.tile([128, 3, UC], F16, tag=f"s{ht}", name="sabg")
                    for ht in range(NHT)]
            g2 = [gpool.tile([128, UC], F16, tag=f"g{ht}", name="g2")
                  for ht in range(NHT)]
            f2u = [gpool.tile([128, UC], F16, tag=f"f{ht}", name="f2")
                   for ht in range(NHT)]
            w2u = [gpool.tile([128, UC], F16, tag=f"w{ht}", name="w2")
                   for ht in range(NHT)]
            v2u = [gpool.tile([128, UC], F16, tag=f"v{ht}", name="v2")
                   for ht in range(NHT)]
            h2u = [hpool.tile([128, UC], F16, tag=f"h{ht}", name="h2")
                   for ht in range(NHT)]
            units[u] = (f2u, v2u, h2u)
            # unit-wide pc: both chunks of the c-projection live at once so
            # sigma_c and g run once per unit at 1024 cols
            pcu = [ppool.tile([128, 2, TC], F32, tag=f"c{ht}", name=f"c{ht}")
                   for ht in range(NHT)]

            # chunk-major emission: the two ht chains interleave in every
            # in-order sequencer queue, so a stalled chunk of one chain never
            # blocks the ready chunk of the other
            for k in range(2):
                ksl = slice(k * TC, (k + 1) * TC)
                pab = [None] * NHT
                for ht in range(NHT):
                    W0 = ht * 10
                    pab[ht] = ppool.tile([128, 2, TC], F32, tag=f"p{ht}",
                                         name=f"p{ht}")
                    for kp in range(2):  # a, b projections
                        nc.tensor.matmul(pab[ht][:, 0, :],
                                         wt[:, W0 + kp, :, :],
                                         x8t[:, kp, :, ksl], start=(kp == 0),
                                         stop=(kp == 1),
                                         perf_mode=PM.DoubleRow)
                        nc.tensor.matmul(pab[ht][:, 1, :],
                                         wt[:, W0 + 2 + kp, :, :],
                                         x8t[:, kp, :, ksl], start=(kp == 0),
                                         stop=(kp == 1),
                                         perf_mode=PM.DoubleRow)
                    # c+0.5: x8@Whi + x8@Elo + xr@Wxr (Wxr row 511 = +32)
                    for j, (wsl, xt) in enumerate([(W0 + 4, x8t),
                                                   (W0 + 6, x8t),
                                                   (W0 + 8, xrt)]):
                        for kp in range(2):
                            nc.tensor.matmul(pcu[ht][:, k, :],
                                             wt[:, wsl + kp, :, :],
                                             xt[:, kp, :, ksl],
                                             start=(j == 0 and kp == 0),
                                             stop=(j == 2 and kp == 1),
                                             perf_mode=PM.DoubleRow)
                for ht in range(NHT):
                    nc.scalar.activation(sabg[ht][:, 0:2, ksl], pab[ht][:],
                                         AF.Sigmoid, scale=1.0 / 64.0)
                if u == 0:
                    # pipeline-fill: run the whole tail per chunk for the
                    # first unit so DVE work starts as early as possible
                    for ht in range(NHT):
                        nc.scalar.activation(sabg[ht][:, 2, ksl],
                                             pcu[ht][:, k, :], AF.Sigmoid,
                                             bias=auxt[:, 2:3],
                                             scale=1.0 / 64.0)
                        nc.vector.scalar_tensor_tensor(
                            g2[ht][:, ksl], pcu[ht][:, k, :], 1.0 / 64.0,
                            sabg[ht][:, 2, ksl], OP.mult, OP.max)
                        nc.vector._custom_dve(FDIV_OP, out=f2u[ht][:, ksl],
                                              in0=sabg[ht][:, 0, ksl],
                                              in1=sabg[ht][:, 1, ksl],
                                              s0=FDIV_CONSTS["s0"],
                                              s1=FDIV_CONSTS["s1"])
                        nc.vector.tensor_scalar(w2u[ht][:, ksl],
                                                f2u[ht][:, ksl], -1.0, 1.0,
                                                OP.mult, OP.add)
                        nc.gpsimd.tensor_tensor(v2u[ht][:, ksl],
                                                w2u[ht][:, ksl],
                                                g2[ht][:, ksl], op=OP.mult)
            if u == 0:
                for ht in range(NHT):
                    pending_scan.append((ht, u))
                for ht in range(NHT):
                    pending_out.append((ht, usl, h2u[ht]))
                continue
            for ht in range(NHT):
                # sigma_c = sigmoid(pc/64 - 0.5) = sigmoid(c), whole unit
                sgc = sabg[ht][:, 2, :].rearrange("p (a b) -> p a b", a=2)
                nc.scalar.activation(sgc, pcu[ht][:], AF.Sigmoid,
                                     bias=auxt[:, 2:3], scale=1.0 / 64.0)
            for ht in range(NHT):
                # per-ht tail emitted as [g, A, w, v] so each ht's v reaches
                # Pool right after its own w, mid-block, with the other ht's
                # DVE work left to overlap the Pool multiply
                g3 = g2[ht][:].rearrange("p (a b) -> p a b", a=2)
                sg3 = sabg[ht][:, 2, :].rearrange("p (a b) -> p a b", a=2)
                # g = max(c + 0.5, sigma_c)   (PSUM read -> DVE)
                nc.vector.scalar_tensor_tensor(g3, pcu[ht][:], 1.0 / 64.0,
                                               sg3, OP.mult, OP.max)
                nc.vector._custom_dve(FDIV_OP, out=f2u[ht][:],
                                      in0=sabg[ht][:, 0, :],
                                      in1=sabg[ht][:, 1, :],
                                      s0=FDIV_CONSTS["s0"],
                                      s1=FDIV_CONSTS["s1"])
                nc.vector.tensor_scalar(w2u[ht][:], f2u[ht][:], -1.0, 1.0,
                                        OP.mult, OP.add)
                if ht == 0:
                    nc.gpsimd.tensor_tensor(v2u[ht][:], w2u[ht][:], g2[ht][:],
                                            op=OP.mult)
                else:
                    # split: first half to Pool (fits before the deferred
                    # scan), second half on DVE
                    nc.gpsimd.tensor_tensor(v2u[ht][:, :TC], w2u[ht][:, :TC],
                                            g2[ht][:, :TC], op=OP.mult)
                    nc.vector.tensor_tensor(v2u[ht][:, TC:], w2u[ht][:, TC:],
                                            g2[ht][:, TC:], op=OP.mult)
                # scans are emitted one unit late so a scan waiting on
                # Pool's v never head-blocks the DVE queue
                pending_scan.append((ht, u))
            while len(pending_scan) > NHT:
                ph, pu = pending_scan.pop(0)
                _emit_scan(ph, pu)
            for ht in range(NHT):
                pending_out.append((ht, usl, h2u[ht]))
            # emit the previous unit's output DMAs here (one unit late, on
            # the ACT queue) so their scan-chain waits never sit in front of
            # the x prefetches or the sigma dispatches in a sequencer queue
            while len(pending_out) > NHT:
                oht, ousl, oh2 = pending_out.pop(0)
                nc.scalar.dma_start(outd[:, oht, ousl], oh2[:])
        while pending_scan:
            ph, pu = pending_scan.pop(0)
            pf, pv, phh = units[pu]
            for k in range(2):
                kq = slice(k * TC, (k + 1) * TC)
                ini = (auxt[:, ph:ph + 1] if (pu == 0 and k == 0)
                       else carry[ph])
                nc.vector.tensor_tensor_scan(phh[ph][:, kq], pf[ph][:, kq],
                                             pv[ph][:, kq], ini,
                                             OP.mult, OP.add)
                carry[ph] = phh[ph][:, (k + 1) * TC - 1:(k + 1) * TC]
        # final unit: split the store per chunk so the drain overlaps
        while pending_out:
            oht, ousl, oh2 = pending_out.pop(0)
            for k in range(2):
                osl = slice(ousl.start + k * TC, ousl.start + (k + 1) * TC)
                nc.scalar.dma_start(outd[:, oht, osl], oh2[:, k * TC:(k + 1) * TC])
    nc.compile()
    return nc


def _get_nc(mm_mode=MM_MODE):
    if mm_mode not in _nc_cache:
        _nc_cache[mm_mode] = _build_nc(mm_mode)
    return _nc_cache[mm_mode]


def _g_host(x):
    # exp(log_g(x)) of the reference, computed directly in fp32
    return np.where(x >= 0, x + 0.5, 1.0 / (1.0 + np.exp(-np.minimum(x, 0))))


def _q8(a):
    return a.astype(NPF8)


def _pack_x(xT_f32):
    """[D, T] f32 -> (x8 pack, xr pack) in [128, kp, i, T] fp8 pair layout;
    xr row d=511 is the constant 1.0 that delivers the c-gate's +32 bias."""
    x8 = _q8(xT_f32)
    xr = _q8(xT_f32 - x8.astype(np.float32))
    xr[511, :] = NPF8(1.0)
    def pack(a):
        return np.ascontiguousarray(
            a.reshape(2, 2, 128, T).transpose(2, 0, 1, 3))
    return pack(x8), pack(xr)


def _pack_w_slices(mat, ht):
    """[512, 256] fp8 -> two [128, 2, 128] pair slices (kp = 0, 1)."""
    r = mat.reshape(2, 2, 128, 2, 128)  # [kp, i, p, ht, m]
    return [np.ascontiguousarray(r[kp, :, :, ht, :].transpose(1, 0, 2))
            for kp in range(2)]


def _run(inputs, mm_mode=MM_MODE, trace=False):
    x = np.asarray(inputs["x"], np.float32)
    h_0 = np.asarray(inputs["h_0"], np.float32)
    W_f = np.asarray(inputs["W_f"], np.float32)
    b_f = np.asarray(inputs["b_f"], np.float32)
    W_i = np.asarray(inputs["W_i"], np.float32)
    b_i = np.asarray(inputs["b_i"], np.float32)
    W_h = np.asarray(inputs["W_h"], np.float32)
    b_h = np.asarray(inputs["b_h"], np.float32)
    assert (b_f == 0).all() and (b_i == 0).all() and (b_h == 0).all(), \
        "device program folds zero biases"

    g0 = _g_host(h_0[:, 0, :])  # [B, H]
    xpacks = [_pack_x(np.ascontiguousarray(x[b].T)) for b in range(B)]

    in_maps = []
    for c in range(NCORES):
        b, hh = divmod(c, 2)
        hs = slice(hh * HS, (hh + 1) * HS)
        wf8 = _q8(WSCALE * W_f[:, hs])
        wi8 = _q8(WSCALE * W_i[:, hs])
        whi = _q8(WSCALE * W_h[:, hs])
        elo = _q8(WSCALE * W_h[:, hs] - whi.astype(np.float32))
        wxr = whi.copy()
        wxr[511, :] = NPF8(32.0)
        wcat = np.zeros((128, 21, 2, 128), NPF8)
        for ht in range(NHT):
            for mi, mat in enumerate((wf8, wi8, whi, elo, wxr)):
                s0, s1 = _pack_w_slices(mat, ht)
                wcat[:, ht * 10 + mi * 2, :, :] = s0
                wcat[:, ht * 10 + mi * 2 + 1, :, :] = s1
        wcat[:, 20, :, :] = NPF8(0.125)
        aux = np.empty((128, 3), np.float32)
        aux[:, 0:2] = g0[b, hs].reshape(2, 128).T
        aux[:, 2] = -0.5
        x8p, xrp = xpacks[b]
        in_maps.append({"x8": x8p, "xr": xrp, "w": wcat, "aux": aux})

    nc = _get_nc(mm_mode)
    res = run_bass_kernel_spmd(nc, in_maps, core_ids=list(range(NCORES)),
                               trace=trace)

    out = np.empty((B, T + 1, H), np.float32)
    out[:, 0, :] = g0
    for c in range(NCORES):
        b, hh = divmod(c, 2)
        hs = slice(hh * HS, (hh + 1) * HS)
        r = np.asarray(res.results[c]["out"], np.float32)  # [128, NHT, T]
        out[b, 1:, hs] = r.transpose(2, 1, 0).reshape(T, HS)
    return out, res


def kernel(**inputs):
    out, _ = _run(inputs)
    return out
